# revision 1
# baseline (speedup 1.0000x reference)
"""MoLoRA linear kernel for Trainium2 (8 NeuronCores, SPMD data-parallel).

Computes: out = x @ W.T + alpha * (per-token top-2 routed LoRA)
Sharding: tokens (B*S = 4096) split 8 ways; all weights replicated.

Numerics: the base matmul runs on the PE array in fp16 hi/lo split form
(x = hi + lo with hi = fp16(x)): out = xh*wh + xh*wl + xl*wh, accumulated
in fp32 PSUM. Measured accuracy vs fp64 is ~4e-7 scale-relative absmax —
indistinguishable from a plain fp32 matmul — at 3 passes of 1 cycle/row
instead of fp32's 4 cycles/row. The LoRA down/up projections use a single
fp16 pass (their contribution to the output is ~1e-6 of scale). Router
logits get the full 3-pass treatment so expert selection matches an fp32
reference. Renormalized top-2 softmax == sigmoid of the top-2 logit gap.

Self-contained: needs numpy + the concourse (bass) stack importable
(falls back to /opt/trn_rl_repo).
"""

import sys

import numpy as np

try:
    import concourse.bass as bass  # noqa: F401
except Exception:  # pragma: no cover
    sys.path.insert(0, "/opt/trn_rl_repo")

import concourse.bacc as bacc
import concourse.mybir as mybir
import concourse.tile as tile
from concourse import bass_utils
from concourse.masks import make_identity

F32 = mybir.dt.float32
F16 = mybir.dt.float16
F8 = mybir.dt.float8e4
F8E5 = mybir.dt.float8e5
AX = mybir.AxisListType.X
OP = mybir.AluOpType

# Problem shapes (hardcoded per contract)
B, S, H, O, E, R = 2, 2048, 2048, 2048, 8, 16
ER = E * R            # 128 = stacked lora rank dim, exactly one partition dim
GA = ER + E           # 136 = lora-A cols + gate cols, fused moving operand
TOKENS = B * S        # 4096
NCORES = 8
T = TOKENS // NCORES  # 512 tokens per core
P = 128
KT = H // P           # 16 contraction chunks
NTC = T // P          # 4 token chunks of 128
KC = 4                # k chunks per weight DMA (512 KB transfers)
KP = 8                # k-pairs for DoubleRow (2 k-chunks each)
XL8_SCALE = 64.0      # xl*64 / wh/64 in fp8: the scales cancel in the product
WL8_SCALE = 512.0     # wl*512 (e4m3) x xh/512 (e5m2): scales cancel likewise
LORA_ALPHA = 16.0
NEG_BIG = 1.0e30


def _build_nc():
    """Build the per-core bass program (identical on all 8 cores)."""
    nc = bacc.Bacc(None, target_bir_lowering=False, debug=False)

    xh = nc.dram_tensor("xh", [H, T], F16, kind="ExternalInput")
    xl = nc.dram_tensor("xl", [H, T], F16, kind="ExternalInput")
    wh = nc.dram_tensor("wh", [H, O], F16, kind="ExternalInput")
    wl = nc.dram_tensor("wl", [H, O], F16, kind="ExternalInput")
    gah = nc.dram_tensor("gah", [H, GA], F16, kind="ExternalInput")
    gal = nc.dram_tensor("gal", [H, GA], F16, kind="ExternalInput")
    bcat = nc.dram_tensor("bcat", [ER, O], F16, kind="ExternalInput")
    xl8 = nc.dram_tensor("xl8", [KP, P, 2, T], F8, kind="ExternalInput")
    wh8 = nc.dram_tensor("wh8", [KP, P, 2, O], F8, kind="ExternalInput")
    xh8 = nc.dram_tensor("xh8", [KP, P, 2, T], F8E5, kind="ExternalInput")
    wl8 = nc.dram_tensor("wl8", [KP, P, 2, O], F8, kind="ExternalInput")
    out = nc.dram_tensor("out", [T, O], F32, kind="ExternalOutput")

    xh_r = xh[:, :].rearrange("(k p) t -> p k t", p=P)
    xl_r = xl[:, :].rearrange("(k p) t -> p k t", p=P)
    gah_r = gah[:, :].rearrange("(k p) g -> p k g", p=P)
    gal_r = gal[:, :].rearrange("(k p) g -> p k g", p=P)
    wh_r = wh[:, :].rearrange("(k p) o -> p k o", p=P)
    wl_r = wl[:, :].rearrange("(k p) o -> p k o", p=P)
    xl8_r = xl8[:, :, :, :].rearrange("c p i t -> p c i t")
    wh8_r = wh8[:, :, :, :].rearrange("c p i o -> p c i o")
    xh8_r = xh8[:, :, :, :].rearrange("c p i t -> p c i t")
    wl8_r = wl8[:, :, :, :].rearrange("c p i o -> p c i o")

    with tile.TileContext(nc) as tc:
        with (
            tc.tile_pool(name="const", bufs=1) as const_pool,
            tc.tile_pool(name="big", bufs=1) as big_pool,
            tc.tile_pool(name="wstream", bufs=3) as w_pool,
            tc.tile_pool(name="ostage", bufs=4) as o_pool,
            tc.tile_pool(name="router", bufs=1) as r_pool,
            tc.tile_pool(name="psum", bufs=1, space="PSUM") as pp,
        ):
            identity = const_pool.tile([P, P], F16)
            make_identity(nc, identity)

            # ---- resident loads (ACT HWDGE ring; weight stream uses SP) ----
            # Quarter-0 sweep 1 needs only xh + gah first; xl/gal/bcat follow.
            # ACT ring: gah then the x stream; SP ring: gal then weights.
            # k=0's prerequisites (gah, gal, xh[0:2], wh_res[0:4], wl[0:4])
            # land in parallel across both rings right after the preamble.
            xh_sb = big_pool.tile([P, KT, T], F16)
            xl_sb = big_pool.tile([P, KT, T], F16)
            nc.scalar.dma_start(out=xh_sb[:, 0:2, :], in_=xh_r[:, 0:2, :])
            gah_sb = big_pool.tile([P, KT, GA], F16)
            nc.scalar.dma_start(out=gah_sb[:], in_=gah_r[:])
            gal_sb = big_pool.tile([P, KT, GA], F16)
            for lo, hi in [(2, 4), (4, 8), (8, 12), (12, 16)]:
                nc.scalar.dma_start(out=xh_sb[:, lo:hi, :], in_=xh_r[:, lo:hi, :])
            for lo, hi in [(0, 4), (4, 8), (8, 12), (12, 16)]:
                nc.scalar.dma_start(out=xl_sb[:, lo:hi, :], in_=xl_r[:, lo:hi, :])
            xl8_sb = big_pool.tile([P, KP, 2, T], F8)
            nc.scalar.dma_start(out=xl8_sb[:], in_=xl8_r[:])
            xh8_sb = big_pool.tile([P, KP, 2, T], F8E5)
            nc.scalar.dma_start(out=xh8_sb[:], in_=xh8_r[:])
            bcat_sb = big_pool.tile([P, O], F16)
            nc.scalar.dma_start(out=bcat_sb[:], in_=bcat[:, :])

            twT_sb = big_pool.tile([P, T], F16)   # weighted lora-down, [er, t]

            def quarter0(ga_tiles):
                """O-quarter 0 (banks pb0-3) + the phase-2 matmuls (pb4-7).

                Two sweeps: xh vs (resident wh, streamed wl) + lora-down/
                logits hi-passes; then xl vs resident wh + logits lo-pass.
                The lo operands stay off the critical DMA path during rampup.
                """
                OQ = 512
                cols = slice(0, OQ)
                accs = [
                    pp.tile([P, OQ], F32, name=f"acc0_{i}", tag=f"pb{i}")
                    for i in range(NTC)
                ]
                wh_res = big_pool.tile([P, KT, OQ], F16)

                def ga_hi_mms(k):
                    for i in range(NTC):
                        ts = slice(i * P, (i + 1) * P)
                        nc.tensor.matmul(
                            ga_tiles[i][:], lhsT=xh_sb[:, k, ts],
                            rhs=gah_sb[:, k, :], start=(k == 0), stop=False,
                        )
                        nc.tensor.matmul(
                            ga_tiles[i][:, ER:GA], lhsT=xh_sb[:, k, ts],
                            rhs=gal_sb[:, k, ER:GA], start=False, stop=False,
                        )

                # sweep-2's fp8 operands, prefetched during sweep 1 so the
                # sync ring is EMPTY when quarter 1's weight stream starts
                w8_tiles, wl8_tiles = [], []

                # the ga matmuls trail the base ones by KC chunks so the gate
                # tensors (gal lands after the first weight chunks) never
                # block the in-order PE queue during DMA rampup
                for kc in range(KT // KC):
                    ks = slice(kc * KC, (kc + 1) * KC)
                    if kc == 0:
                        nc.sync.dma_start(out=wh_res[:, 0:2, :],
                                          in_=wh_r[:, 0:2, cols])
                        nc.sync.dma_start(out=wh_res[:, 2:4, :],
                                          in_=wh_r[:, 2:4, cols])
                        nc.sync.dma_start(out=gal_sb[:], in_=gal_r[:])
                    else:
                        nc.sync.dma_start(out=wh_res[:, ks, :],
                                          in_=wh_r[:, ks, cols])
                    for c in (2 * kc, 2 * kc + 1):
                        w8_t = w_pool.tile([P, 2, OQ], F8, name="w8_t",
                                           tag="w8_t", bufs=8)
                        nc.sync.dma_start(out=w8_t[:], in_=wh8_r[:, c, :, cols])
                        w8_tiles.append(w8_t)
                        wl8_t = w_pool.tile([P, 2, OQ], F8, name="wl8_t",
                                            tag="wl8_t", bufs=8)
                        nc.sync.dma_start(out=wl8_t[:], in_=wl8_r[:, c, :, cols])
                        wl8_tiles.append(wl8_t)
                    for kk in range(KC):
                        k = kc * KC + kk
                        for i in range(NTC):
                            ts = slice(i * P, (i + 1) * P)
                            nc.tensor.matmul(
                                accs[i][:], lhsT=xh_sb[:, k, ts],
                                rhs=wh_res[:, k, :], start=(k == 0), stop=False,
                            )
                        if kc > 0:
                            ga_hi_mms(k - KC)
                for k in range(KT - KC, KT):
                    ga_hi_mms(k)
                for c in range(KP):
                    for kk in range(2):
                        k = 2 * c + kk
                        for i in range(NTC):
                            ts = slice(i * P, (i + 1) * P)
                            nc.tensor.matmul(
                                ga_tiles[i][:, ER:GA], lhsT=xl_sb[:, k, ts],
                                rhs=gah_sb[:, k, ER:GA], start=False,
                                stop=(k == KT - 1),
                            )
                    for i in range(NTC):
                        ts = slice(i * P, (i + 1) * P)
                        nc.tensor.matmul(
                            accs[i][:], lhsT=xl8_sb[:, c, :, ts],
                            rhs=w8_tiles[c][:], start=False, stop=False,
                            perf_mode=mybir.MatmulPerfMode.DoubleRow,
                        )
                        nc.tensor.matmul(
                            accs[i][:], lhsT=xh8_sb[:, c, :, ts],
                            rhs=wl8_tiles[c][:], start=False, stop=False,
                            perf_mode=mybir.MatmulPerfMode.DoubleRow,
                        )
                return accs

            def base_quarter(q, up_first):
                """One O-quarter of the base matmul; banks alternate between
                pb0-3 (even q) and pb4-7 (odd q) so a quarter can start while
                the previous one drains. If up_first, the lora up-projection
                opens each accumulation group (twT must already be ready)."""
                OQ = 512
                cols = slice(q * OQ, (q + 1) * OQ)
                bank = (q % 2) * 4
                accs = [
                    pp.tile([P, OQ], F32, name=f"acc{q}_{i}", tag=f"pb{bank + i}")
                    for i in range(NTC)
                ]
                if up_first:
                    for i in range(NTC):
                        ts = slice(i * P, (i + 1) * P)
                        nc.tensor.matmul(
                            accs[i][:], lhsT=twT_sb[:, ts],
                            rhs=bcat_sb[:, cols], start=True, stop=False,
                        )
                for kc in range(KT // KC):
                    ks = slice(kc * KC, (kc + 1) * KC)
                    wh_t = w_pool.tile([P, KC, OQ], F16, name="wh_t", tag="wh_t")
                    nc.sync.dma_start(out=wh_t[:], in_=wh_r[:, ks, cols])
                    w8_t = w_pool.tile([P, 2, 2, OQ], F8, name="w8q_t", tag="w8q_t")
                    wl8_t = w_pool.tile([P, 2, 2, OQ], F8, name="wl8q_t", tag="wl8q_t")
                    for cc in range(2):
                        nc.sync.dma_start(
                            out=w8_t[:, cc, :, :],
                            in_=wh8_r[:, 2 * kc + cc, :, cols],
                        )
                        nc.sync.dma_start(
                            out=wl8_t[:, cc, :, :],
                            in_=wl8_r[:, 2 * kc + cc, :, cols],
                        )
                    for kk in range(KC):
                        k = kc * KC + kk
                        for i in range(NTC):
                            ts = slice(i * P, (i + 1) * P)
                            nc.tensor.matmul(
                                accs[i][:], lhsT=xh_sb[:, k, ts],
                                rhs=wh_t[:, kk, :],
                                start=(k == 0 and not up_first), stop=False,
                            )
                    for cc in range(2):
                        c = 2 * kc + cc
                        for i in range(NTC):
                            ts = slice(i * P, (i + 1) * P)
                            nc.tensor.matmul(
                                accs[i][:], lhsT=xl8_sb[:, c, :, ts],
                                rhs=w8_t[:, cc, :, :], start=False, stop=False,
                                perf_mode=mybir.MatmulPerfMode.DoubleRow,
                            )
                            nc.tensor.matmul(
                                accs[i][:], lhsT=xh8_sb[:, c, :, ts],
                                rhs=wl8_t[:, cc, :, :], start=False,
                                stop=(kc == KT // KC - 1 and cc == 1 and up_first),
                                perf_mode=mybir.MatmulPerfMode.DoubleRow,
                            )
                return accs

            def up_close(q, accs):
                """Close each accumulation group with the lora up matmul."""
                OQ = 512
                for i in range(NTC):
                    ts = slice(i * P, (i + 1) * P)
                    nc.tensor.matmul(
                        accs[i][:], lhsT=twT_sb[:, ts],
                        rhs=bcat_sb[:, q * OQ : (q + 1) * OQ],
                        start=False, stop=True,
                    )

            def evict(q, accs):
                OQ = 512
                for i in range(NTC):
                    o_t = o_pool.tile([P, OQ], F32, name="o_t", tag="o_t")
                    # DVE copies only: ACT must stay free to trigger its
                    # HWDGE DMA ring without queueing behind slow copies
                    nc.vector.tensor_copy(o_t[:], accs[i][:])
                    nc.sync.dma_start(
                        out=out[i * P : (i + 1) * P, q * OQ : (q + 1) * OQ],
                        in_=o_t[:],
                    )

            def router_math(ga_tiles):
                """Batched top-2 routing for all 4 token chunks at once.
                ga_tiles[i][:, ER:GA] are the logits [t=128, e=8]."""
                l_all = r_pool.tile([P, NTC, E], F32, name="l_all")
                for i in range(NTC):
                    nc.vector.tensor_copy(l_all[:, i, :], ga_tiles[i][:, ER:GA])
                m1 = r_pool.tile([P, NTC], F32, name="m1")
                nc.vector.reduce_max(out=m1[:], in_=l_all[:], axis=AX)

                def bcast(ap):  # [P, NTC] -> [P, NTC, E]
                    return ap.rearrange("p c -> p c ()").broadcast_to([P, NTC, E])

                is1 = r_pool.tile([P, NTC, E], F32, name="is1")
                nc.vector.tensor_tensor(
                    out=is1[:], in0=l_all[:], in1=bcast(m1[:]), op=OP.is_equal
                )
                l2 = r_pool.tile([P, NTC, E], F32, name="l2")
                nc.vector.tensor_scalar(
                    out=l2[:], in0=is1[:], scalar1=-NEG_BIG, scalar2=None,
                    op0=OP.mult,
                )
                nc.vector.tensor_add(out=l2[:], in0=l2[:], in1=l_all[:])
                m2 = r_pool.tile([P, NTC], F32, name="m2")
                nc.vector.reduce_max(out=m2[:], in_=l2[:], axis=AX)
                is2 = r_pool.tile([P, NTC, E], F32, name="is2")
                nc.vector.tensor_tensor(
                    out=is2[:], in0=l2[:], in1=bcast(m2[:]), op=OP.is_equal
                )
                # s1 = sigmoid(m1 - m2) on ACT; s2 = 1 - s1 via sigmoid(-d)
                d12 = r_pool.tile([P, NTC], F32, name="d12")
                nc.vector.tensor_sub(out=d12[:], in0=m1[:], in1=m2[:])
                s1 = r_pool.tile([P, NTC], F32, name="s1")
                nc.scalar.activation(s1[:], d12[:], mybir.ActivationFunctionType.Sigmoid)
                s2 = r_pool.tile([P, NTC], F32, name="s2")
                nc.scalar.activation(
                    s2[:], d12[:], mybir.ActivationFunctionType.Sigmoid, scale=-1.0
                )
                cw = r_pool.tile([P, NTC, E], F32, name="cw")
                nc.vector.tensor_tensor(
                    out=cw[:], in0=is1[:], in1=bcast(s1[:]), op=OP.mult
                )
                cw2 = r_pool.tile([P, NTC, E], F32, name="cw2")
                nc.vector.tensor_tensor(
                    out=cw2[:], in0=is2[:], in1=bcast(s2[:]), op=OP.mult
                )
                nc.vector.tensor_add(out=cw[:], in0=cw[:], in1=cw2[:])

                # tw[t, (e r)] = t_down[t, (e r)] * cw[t, e]; transpose to
                # [er, t] for use as the up-projection stationary operand.
                for i in range(NTC):
                    ts = slice(i * P, (i + 1) * P)
                    tw_sb = r_pool.tile([P, ER], F16, name=f"tw_sb{i}", tag="tw_sb")
                    nc.vector.tensor_tensor(
                        out=tw_sb[:].rearrange("p (e r) -> p e r", r=R),
                        in0=ga_tiles[i][:, 0:ER].rearrange("p (e r) -> p e r", r=R),
                        in1=cw[:, i, :].rearrange("p e -> p e ()").broadcast_to(
                            [P, E, R]
                        ),
                        op=OP.mult,
                    )
                    twT_ps = pp.tile([P, P], F16, name=f"twT_ps{i}", tag=f"pb{4 + i}")
                    nc.tensor.transpose(twT_ps[:], tw_sb[:], identity[:])
                    nc.vector.tensor_copy(twT_sb[:, ts], twT_ps[:])

            # ---- program ----
            # ga_ps[t, 0:128] = lora-down t (fp16 hi pass only: plenty);
            # ga_ps[t, 128:136] = router logits (hi*hi + hi*lo + lo*hi).
            ga_tiles = [
                pp.tile([P, GA], F32, name=f"ga_ps{i}", tag=f"pb{4 + i}")
                for i in range(NTC)
            ]
            accs0 = quarter0(ga_tiles)
            router_math(ga_tiles)                # DVE/ACT; frees pb4-7
            accs1 = base_quarter(1, up_first=False)  # pb4-7, starts right away
            up_close(0, accs0)                   # twT ready by now
            evict(0, accs0)
            accs2 = base_quarter(2, up_first=True)   # pb0-3 after q0 evict
            up_close(1, accs1)
            evict(1, accs1)
            accs3 = base_quarter(3, up_first=True)   # pb4-7 after q1 evict
            evict(2, accs2)
            evict(3, accs3)

    nc.compile()
    return nc


_NC_CACHE = {}


def _get_nc():
    if "nc" not in _NC_CACHE:
        _NC_CACHE["nc"] = _build_nc()
    return _NC_CACHE["nc"]


def _split16(a):
    hi = a.astype(np.float16)
    lo = (a - hi.astype(np.float32)).astype(np.float16)
    return np.ascontiguousarray(hi), np.ascontiguousarray(lo)


def _prep_in_maps(x, weight, gate_w, A_w, B_w):
    xf = np.asarray(x, np.float32).reshape(TOKENS, H)
    wT = np.ascontiguousarray(np.asarray(weight, np.float32).T)
    acatT = np.asarray(A_w, np.float32).transpose(2, 0, 1).reshape(H, ER)
    gacatT = np.ascontiguousarray(
        np.concatenate([acatT, np.asarray(gate_w, np.float32).T], axis=1)
    )
    bcat = np.ascontiguousarray(
        (np.asarray(B_w, np.float32).transpose(0, 2, 1).reshape(ER, O) * LORA_ALPHA)
        .astype(np.float16)
    )
    wh, wl = _split16(wT)
    gah, gal = _split16(gacatT)
    import ml_dtypes

    def to_fp8_pairs(a, scale, dt8=None):
        # [H, N] fp16 -> [KP, P, 2, N] fp8 with k-chunk pairs interleaved
        if dt8 is None:
            dt8 = ml_dtypes.float8_e4m3
        return np.ascontiguousarray(
            (a.astype(np.float32) * scale)
            .astype(dt8)
            .reshape(KP, 2, P, a.shape[1])
            .transpose(0, 2, 1, 3)
        )

    wh8 = to_fp8_pairs(wh, 1.0 / XL8_SCALE)
    wl8 = to_fp8_pairs(wl, WL8_SCALE)
    shared = {"wh": wh, "wl": wl, "gah": gah, "gal": gal, "bcat": bcat,
              "wh8": wh8, "wl8": wl8}
    in_maps = []
    for c in range(NCORES):
        xTc = np.ascontiguousarray(xf[c * T : (c + 1) * T, :].T)
        xch, xcl = _split16(xTc)
        in_maps.append({"xh": xch, "xl": xcl,
                        "xl8": to_fp8_pairs(xcl, XL8_SCALE),
                        "xh8": to_fp8_pairs(xch, 1.0 / WL8_SCALE,
                                            ml_dtypes.float8_e5m2),
                        **shared})
    return in_maps


def kernel(x, weight, gate_w, A_w, B_w, _trace=False, **_ignored):
    in_maps = _prep_in_maps(x, weight, gate_w, A_w, B_w)
    nc = _get_nc()
    res = bass_utils.run_bass_kernel_spmd(
        nc, in_maps, core_ids=list(range(NCORES)), trace=_trace
    )
    outs = [res.results[c]["out"] for c in range(NCORES)]
    full = np.concatenate(outs, axis=0).reshape(B, S, O).astype(np.float32)
    if _trace:
        kernel.last_result = res
    return full



# revision 2
# speedup vs baseline: 1.7326x; 1.7326x over previous
"""MoLoRA linear kernel for Trainium2 (8 NeuronCores, SPMD data-parallel).

Computes: out = x @ W.T + alpha * (per-token top-2 routed LoRA)
Sharding: tokens (B*S = 4096) split 8 ways; all weights replicated.

Numerics: everything runs as a SINGLE fp16 pass on the PE array with fp32
PSUM accumulation. fp16 input quantization gives ~3e-4 relative RMS error
on this problem (numpy-simulated end to end, zero expert flips) against a
2e-2 gate — no hi/lo split or fp8 correction passes needed. Router logits
in fp16 shift expert selection only for top2/top3 logit gaps < ~2e-3,
and a flipped expert perturbs only the (1%-of-magnitude) LoRA term.
Renormalized top-2 softmax == sigmoid of the top-2 logit gap.

Self-contained: needs numpy + the concourse (bass) stack importable
(falls back to /opt/trn_rl_repo).
"""

import sys

import numpy as np

try:
    import concourse.bass as bass  # noqa: F401
except Exception:  # pragma: no cover
    sys.path.insert(0, "/opt/trn_rl_repo")

import concourse.bacc as bacc
import concourse.mybir as mybir
import concourse.tile as tile
from concourse import bass_utils
from concourse.masks import make_identity

F32 = mybir.dt.float32
F16 = mybir.dt.float16
AX = mybir.AxisListType.X
OP = mybir.AluOpType

# Problem shapes (hardcoded per contract)
B, S, H, O, E, R = 2, 2048, 2048, 2048, 8, 16
ER = E * R            # 128 = stacked lora rank dim, exactly one partition dim
GA = ER + E           # 136 = lora-A cols + gate cols, fused moving operand
TOKENS = B * S        # 4096
NCORES = 8
T = TOKENS // NCORES  # 512 tokens per core
P = 128
KT = H // P           # 16 contraction chunks
NTC = T // P          # 4 token chunks of 128
KC = 4                # k chunks per weight DMA (512 KB transfers)
OQ = 512              # output quarter width (one PSUM bank)
LORA_ALPHA = 16.0
NEG_BIG = 1.0e30


def _build_nc():
    """Build the per-core bass program (identical on all 8 cores)."""
    nc = bacc.Bacc(None, target_bir_lowering=False, debug=False)

    xh = nc.dram_tensor("xh", [H, T], F16, kind="ExternalInput")
    wh = nc.dram_tensor("wh", [H, O], F16, kind="ExternalInput")
    gah = nc.dram_tensor("gah", [H, GA], F16, kind="ExternalInput")
    bcat = nc.dram_tensor("bcat", [ER, O], F16, kind="ExternalInput")
    out = nc.dram_tensor("out", [T, O], F16, kind="ExternalOutput")

    xh_r = xh[:, :].rearrange("(k p) t -> p k t", p=P)
    gah_r = gah[:, :].rearrange("(k p) g -> p k g", p=P)
    wh_r = wh[:, :].rearrange("(k p) o -> p k o", p=P)

    with tile.TileContext(nc) as tc:
        with (
            tc.tile_pool(name="const", bufs=1) as const_pool,
            tc.tile_pool(name="big", bufs=1) as big_pool,
            tc.tile_pool(name="wstream", bufs=3) as w_pool,
            tc.tile_pool(name="ostage", bufs=4) as o_pool,
            tc.tile_pool(name="router", bufs=1) as r_pool,
            tc.tile_pool(name="psum", bufs=1, space="PSUM") as pp,
        ):
            identity = const_pool.tile([P, P], F16)
            make_identity(nc, identity)

            # ---- resident loads (ACT HWDGE ring; weight stream uses SP) ----
            # k=0's prerequisites (xh[0:2], wh chunk 0) land first on their
            # respective rings; gah follows for the ga matmuls that start
            # one KC-chunk into quarter 0.
            xh_sb = big_pool.tile([P, KT, T], F16)
            nc.scalar.dma_start(out=xh_sb[:, 0:2, :], in_=xh_r[:, 0:2, :])
            gah_sb = big_pool.tile([P, KT, GA], F16)
            nc.scalar.dma_start(out=gah_sb[:], in_=gah_r[:])
            for lo, hi in [(2, 4), (4, 8), (8, 12), (12, 16)]:
                nc.scalar.dma_start(out=xh_sb[:, lo:hi, :], in_=xh_r[:, lo:hi, :])
            bcat_sb = big_pool.tile([P, O], F16)
            nc.scalar.dma_start(out=bcat_sb[:], in_=bcat[:, :])

            twT_sb = big_pool.tile([P, T], F16)   # weighted lora-down, [er, t]

            def quarter0(ga_tiles):
                """O-quarter 0 (banks pb0-3) with the ga matmuls (pb4-7)
                interleaved so they finish ~75% through the quarter: the
                router chain then overlaps quarter 0's tail and the twT
                transposes issue with no PE stall."""
                cols = slice(0, OQ)
                accs = [
                    pp.tile([P, OQ], F32, name=f"acc0_{i}", tag=f"pb{i}")
                    for i in range(NTC)
                ]

                def ga_mm(k):
                    for i in range(NTC):
                        ts = slice(i * P, (i + 1) * P)
                        nc.tensor.matmul(
                            ga_tiles[i][:], lhsT=xh_sb[:, k, ts],
                            rhs=gah_sb[:, k, :], start=(k == 0),
                            stop=(k == KT - 1),
                        )

                for kc in range(KT // KC):
                    ks = slice(kc * KC, (kc + 1) * KC)
                    wh_t = w_pool.tile([P, KC, OQ], F16, name="wh_t", tag="wh_t")
                    if kc == 0:
                        nc.sync.dma_start(out=wh_t[:, 0:2, :],
                                          in_=wh_r[:, 0:2, cols])
                        nc.sync.dma_start(out=wh_t[:, 2:4, :],
                                          in_=wh_r[:, 2:4, cols])
                    else:
                        nc.sync.dma_start(out=wh_t[:], in_=wh_r[:, ks, cols])
                    for kk in range(KC):
                        k = kc * KC + kk
                        for i in range(NTC):
                            ts = slice(i * P, (i + 1) * P)
                            nc.tensor.matmul(
                                accs[i][:], lhsT=xh_sb[:, k, ts],
                                rhs=wh_t[:, kk, :], start=(k == 0), stop=False,
                            )
                        # ga trails base by one KC chunk (DMA rampup), then
                        # runs 2 k's per base k to finish at base k=11.
                        if kc in (1, 2):
                            ga_mm(2 * (k - KC))
                            ga_mm(2 * (k - KC) + 1)
                return accs

            def base_quarter(q, up_first):
                """One O-quarter of the base matmul; banks alternate between
                pb0-3 (even q) and pb4-7 (odd q) so a quarter can start while
                the previous one drains. If up_first, the lora up-projection
                opens each accumulation group (twT must already be ready)."""
                cols = slice(q * OQ, (q + 1) * OQ)
                bank = (q % 2) * 4
                accs = [
                    pp.tile([P, OQ], F32, name=f"acc{q}_{i}", tag=f"pb{bank + i}")
                    for i in range(NTC)
                ]
                if up_first:
                    for i in range(NTC):
                        ts = slice(i * P, (i + 1) * P)
                        nc.tensor.matmul(
                            accs[i][:], lhsT=twT_sb[:, ts],
                            rhs=bcat_sb[:, cols], start=True, stop=False,
                        )
                for kc in range(KT // KC):
                    ks = slice(kc * KC, (kc + 1) * KC)
                    wh_t = w_pool.tile([P, KC, OQ], F16, name="wh_t", tag="wh_t")
                    nc.sync.dma_start(out=wh_t[:], in_=wh_r[:, ks, cols])
                    for kk in range(KC):
                        k = kc * KC + kk
                        for i in range(NTC):
                            ts = slice(i * P, (i + 1) * P)
                            nc.tensor.matmul(
                                accs[i][:], lhsT=xh_sb[:, k, ts],
                                rhs=wh_t[:, kk, :],
                                start=(k == 0 and not up_first),
                                stop=(k == KT - 1 and up_first),
                            )
                return accs

            def up_close(q, accs):
                """Close each accumulation group with the lora up matmul."""
                for i in range(NTC):
                    ts = slice(i * P, (i + 1) * P)
                    nc.tensor.matmul(
                        accs[i][:], lhsT=twT_sb[:, ts],
                        rhs=bcat_sb[:, q * OQ : (q + 1) * OQ],
                        start=False, stop=True,
                    )

            def evict(q, accs):
                for i in range(NTC):
                    o_t = o_pool.tile([P, OQ], F16, name="o_t", tag="o_t")
                    # DVE copies only: ACT must stay free to trigger its
                    # HWDGE DMA ring without queueing behind slow copies
                    nc.vector.tensor_copy(o_t[:], accs[i][:])
                    nc.scalar.dma_start(
                        out=out[i * P : (i + 1) * P, q * OQ : (q + 1) * OQ],
                        in_=o_t[:],
                    )

            def router_math(ga_tiles):
                """Batched top-2 routing for all 4 token chunks at once.
                ga_tiles[i][:, ER:GA] are the logits [t=128, e=8]."""
                l_all = r_pool.tile([P, NTC, E], F32, name="l_all")
                for i in range(NTC):
                    nc.vector.tensor_copy(l_all[:, i, :], ga_tiles[i][:, ER:GA])
                m1 = r_pool.tile([P, NTC], F32, name="m1")
                nc.vector.reduce_max(out=m1[:], in_=l_all[:], axis=AX)

                def bcast(ap):  # [P, NTC] -> [P, NTC, E]
                    return ap.rearrange("p c -> p c ()").broadcast_to([P, NTC, E])

                is1 = r_pool.tile([P, NTC, E], F32, name="is1")
                nc.vector.tensor_tensor(
                    out=is1[:], in0=l_all[:], in1=bcast(m1[:]), op=OP.is_equal
                )
                l2 = r_pool.tile([P, NTC, E], F32, name="l2")
                nc.vector.tensor_scalar(
                    out=l2[:], in0=is1[:], scalar1=-NEG_BIG, scalar2=None,
                    op0=OP.mult,
                )
                nc.vector.tensor_add(out=l2[:], in0=l2[:], in1=l_all[:])
                m2 = r_pool.tile([P, NTC], F32, name="m2")
                nc.vector.reduce_max(out=m2[:], in_=l2[:], axis=AX)
                is2 = r_pool.tile([P, NTC, E], F32, name="is2")
                nc.vector.tensor_tensor(
                    out=is2[:], in0=l2[:], in1=bcast(m2[:]), op=OP.is_equal
                )
                # s1 = sigmoid(m1 - m2) on ACT; s2 = 1 - s1 via sigmoid(-d)
                d12 = r_pool.tile([P, NTC], F32, name="d12")
                nc.vector.tensor_sub(out=d12[:], in0=m1[:], in1=m2[:])
                s1 = r_pool.tile([P, NTC], F32, name="s1")
                nc.scalar.activation(s1[:], d12[:], mybir.ActivationFunctionType.Sigmoid)
                s2 = r_pool.tile([P, NTC], F32, name="s2")
                nc.scalar.activation(
                    s2[:], d12[:], mybir.ActivationFunctionType.Sigmoid, scale=-1.0
                )
                cw = r_pool.tile([P, NTC, E], F32, name="cw")
                nc.vector.tensor_tensor(
                    out=cw[:], in0=is1[:], in1=bcast(s1[:]), op=OP.mult
                )
                cw2 = r_pool.tile([P, NTC, E], F32, name="cw2")
                nc.vector.tensor_tensor(
                    out=cw2[:], in0=is2[:], in1=bcast(s2[:]), op=OP.mult
                )
                nc.vector.tensor_add(out=cw[:], in0=cw[:], in1=cw2[:])

                # tw[t, (e r)] = t_down[t, (e r)] * cw[t, e]; transpose to
                # [er, t] for use as the up-projection stationary operand.
                for i in range(NTC):
                    ts = slice(i * P, (i + 1) * P)
                    tw_sb = r_pool.tile([P, ER], F16, name=f"tw_sb{i}", tag="tw_sb")
                    nc.vector.tensor_tensor(
                        out=tw_sb[:].rearrange("p (e r) -> p e r", r=R),
                        in0=ga_tiles[i][:, 0:ER].rearrange("p (e r) -> p e r", r=R),
                        in1=cw[:, i, :].rearrange("p e -> p e ()").broadcast_to(
                            [P, E, R]
                        ),
                        op=OP.mult,
                    )
                    twT_ps = pp.tile([P, P], F16, name=f"twT_ps{i}", tag=f"pb{4 + i}")
                    nc.tensor.transpose(twT_ps[:], tw_sb[:], identity[:])
                    nc.vector.tensor_copy(twT_sb[:, ts], twT_ps[:])

            # ---- program ----
            # ga_ps[t, 0:128] = lora-down t; ga_ps[t, 128:136] = router logits.
            ga_tiles = [
                pp.tile([P, GA], F32, name=f"ga_ps{i}", tag=f"pb{4 + i}")
                for i in range(NTC)
            ]
            accs0 = quarter0(ga_tiles)
            router_math(ga_tiles)                # DVE/ACT; frees pb4-7
            accs1 = base_quarter(1, up_first=False)  # pb4-7, starts right away
            up_close(0, accs0)                   # twT ready by now
            evict(0, accs0)
            accs2 = base_quarter(2, up_first=True)   # pb0-3 after q0 evict
            up_close(1, accs1)
            evict(1, accs1)
            accs3 = base_quarter(3, up_first=True)   # pb4-7 after q1 evict
            evict(2, accs2)
            evict(3, accs3)

    nc.compile()
    return nc


_NC_CACHE = {}


def _get_nc():
    if "nc" not in _NC_CACHE:
        _NC_CACHE["nc"] = _build_nc()
    return _NC_CACHE["nc"]


def _prep_in_maps(x, weight, gate_w, A_w, B_w):
    xf = np.asarray(x, np.float32).reshape(TOKENS, H)
    wh = np.ascontiguousarray(np.asarray(weight, np.float32).T).astype(np.float16)
    acatT = np.asarray(A_w, np.float32).transpose(2, 0, 1).reshape(H, ER)
    gah = np.ascontiguousarray(
        np.concatenate([acatT, np.asarray(gate_w, np.float32).T], axis=1)
    ).astype(np.float16)
    bcat = np.ascontiguousarray(
        (np.asarray(B_w, np.float32).transpose(0, 2, 1).reshape(ER, O) * LORA_ALPHA)
        .astype(np.float16)
    )
    shared = {"wh": wh, "gah": gah, "bcat": bcat}
    in_maps = []
    for c in range(NCORES):
        xch = np.ascontiguousarray(xf[c * T : (c + 1) * T, :].T).astype(np.float16)
        in_maps.append({"xh": xch, **shared})
    return in_maps


def kernel(x, weight, gate_w, A_w, B_w, _trace=False, **_ignored):
    in_maps = _prep_in_maps(x, weight, gate_w, A_w, B_w)
    nc = _get_nc()
    res = bass_utils.run_bass_kernel_spmd(
        nc, in_maps, core_ids=list(range(NCORES)), trace=_trace
    )
    outs = [res.results[c]["out"] for c in range(NCORES)]
    full = np.concatenate(outs, axis=0).reshape(B, S, O).astype(np.float32)
    if _trace:
        kernel.last_result = res
    return full


# revision 6
# speedup vs baseline: 1.8265x; 1.0542x over previous
"""MoLoRA linear kernel for Trainium2 (8 NeuronCores, SPMD data-parallel).

Computes: out = x @ W.T + alpha * (per-token top-2 routed LoRA)
Sharding: tokens (B*S = 4096) split 8 ways; all weights replicated.

Numerics: everything runs as a SINGLE fp16 pass on the PE array with fp32
PSUM accumulation. fp16 input quantization gives ~3e-4 relative RMS error
on this problem (numpy-simulated end to end, zero expert flips) against a
2e-2 gate — no hi/lo split or fp8 correction passes needed. Router logits
in fp16 shift expert selection only for top2/top3 logit gaps < ~2e-3,
and a flipped expert perturbs only the (1%-of-magnitude) LoRA term.
Renormalized top-2 softmax == sigmoid of the top-2 logit gap.

Self-contained: needs numpy + the concourse (bass) stack importable
(falls back to /opt/trn_rl_repo).
"""

import sys

import numpy as np

try:
    import concourse.bass as bass  # noqa: F401
except Exception:  # pragma: no cover
    sys.path.insert(0, "/opt/trn_rl_repo")

import concourse.bacc as bacc
import concourse.mybir as mybir
import concourse.tile as tile
from concourse import bass_utils
from concourse.masks import make_identity

F32 = mybir.dt.float32
F16 = mybir.dt.float16
AX = mybir.AxisListType.X
OP = mybir.AluOpType

# Problem shapes (hardcoded per contract)
B, S, H, O, E, R = 2, 2048, 2048, 2048, 8, 16
ER = E * R            # 128 = stacked lora rank dim, exactly one partition dim
GA = ER + E           # 136 = lora-A cols + gate cols, fused moving operand
TOKENS = B * S        # 4096
NCORES = 8
T = TOKENS // NCORES  # 512 tokens per core
P = 128
KT = H // P           # 16 contraction chunks
NTC = T // P          # 4 token chunks of 128
KC = 4                # k chunks per weight DMA (512 KB transfers)
OQ = 512              # output quarter width (one PSUM bank)
LORA_ALPHA = 16.0
NEG_BIG = 1.0e30


def _build_nc():
    """Build the per-core bass program (identical on all 8 cores)."""
    nc = bacc.Bacc(None, target_bir_lowering=False, debug=False)

    xh = nc.dram_tensor("xh", [H, T], F16, kind="ExternalInput")
    wh = nc.dram_tensor("wh", [H, O], F16, kind="ExternalInput")
    gah = nc.dram_tensor("gah", [H, GA], F16, kind="ExternalInput")
    bcat = nc.dram_tensor("bcat", [ER, O], F16, kind="ExternalInput")
    out = nc.dram_tensor("out", [T, O], F16, kind="ExternalOutput")

    xh_r = xh[:, :].rearrange("(k p) t -> p k t", p=P)
    gah_r = gah[:, :].rearrange("(k p) g -> p k g", p=P)
    wh_r = wh[:, :].rearrange("(k p) o -> p k o", p=P)

    with tile.TileContext(nc) as tc:
        with (
            tc.tile_pool(name="const", bufs=1) as const_pool,
            tc.tile_pool(name="big", bufs=1) as big_pool,
            tc.tile_pool(name="wstream", bufs=3) as w_pool,
            tc.tile_pool(name="ostage", bufs=4) as o_pool,
            tc.tile_pool(name="router", bufs=1) as r_pool,
            tc.tile_pool(name="psum", bufs=1, space="PSUM") as pp,
        ):
            identity = const_pool.tile([P, P], F16)
            make_identity(nc, identity)

            # ---- resident loads: xh on the ACT ring, gah/bcat on the (idle)
            # GpSimd ring so they don't queue behind xh; weights stream on SP.
            # First chunks are small so the PE can start ASAP.
            xh_sb = big_pool.tile([P, KT, T], F16)
            nc.scalar.dma_start(out=xh_sb[:, 0:1, :], in_=xh_r[:, 0:1, :])
            nc.scalar.dma_start(out=xh_sb[:, 1:2, :], in_=xh_r[:, 1:2, :])
            gah_sb = big_pool.tile([P, KT, GA], F16)
            nc.gpsimd.dma_start(out=gah_sb[:], in_=gah_r[:])
            for lo, hi in [(2, 4), (4, 8), (8, 12), (12, 16)]:
                nc.scalar.dma_start(out=xh_sb[:, lo:hi, :], in_=xh_r[:, lo:hi, :])
            bcat_sb = big_pool.tile([P, O], F16)
            nc.gpsimd.dma_start(out=bcat_sb[:], in_=bcat[:, :])

            twT_sb = big_pool.tile([P, T], F16)   # weighted lora-down, [er, t]

            def quarter0(ga_tiles):
                """O-quarter 0 (banks pb0-3) with the ga matmuls (pb4-7)
                interleaved so they finish ~75% through the quarter: the
                router chain then overlaps quarter 0's tail and the twT
                transposes issue with no PE stall."""
                cols = slice(0, OQ)
                accs = [
                    pp.tile([P, OQ], F32, name=f"acc0_{i}", tag=f"pb{i}")
                    for i in range(NTC)
                ]

                def ga_mm(k):
                    for i in range(NTC):
                        ts = slice(i * P, (i + 1) * P)
                        nc.tensor.matmul(
                            ga_tiles[i][:], lhsT=xh_sb[:, k, ts],
                            rhs=gah_sb[:, k, :], start=(k == 0),
                            stop=(k == KT - 1),
                        )

                for kc in range(KT // KC):
                    ks = slice(kc * KC, (kc + 1) * KC)
                    wh_t = w_pool.tile([P, KC, OQ], F16, name="wh_t", tag="wh_t")
                    if kc == 0:
                        nc.sync.dma_start(out=wh_t[:, 0:1, :],
                                          in_=wh_r[:, 0:1, cols])
                        nc.sync.dma_start(out=wh_t[:, 1:2, :],
                                          in_=wh_r[:, 1:2, cols])
                        nc.sync.dma_start(out=wh_t[:, 2:4, :],
                                          in_=wh_r[:, 2:4, cols])
                    else:
                        nc.sync.dma_start(out=wh_t[:], in_=wh_r[:, ks, cols])
                    for kk in range(KC):
                        k = kc * KC + kk
                        for i in range(NTC):
                            ts = slice(i * P, (i + 1) * P)
                            nc.tensor.matmul(
                                accs[i][:], lhsT=xh_sb[:, k, ts],
                                rhs=wh_t[:, kk, :], start=(k == 0), stop=False,
                            )
                        # ga trails base by one KC chunk (DMA rampup), then
                        # runs 2 k's per base k to finish at base k=11.
                        if kc in (1, 2):
                            ga_mm(2 * (k - KC))
                            ga_mm(2 * (k - KC) + 1)
                return accs

            def base_quarter(q, up_first, extra_dmas=None):
                """One O-quarter of the base matmul; banks alternate between
                pb0-3 (even q) and pb4-7 (odd q) so a quarter can start while
                the previous one drains. If up_first, the lora up-projection
                opens each accumulation group (twT must already be ready).
                extra_dmas: {kc: fn} — interleave foreign DMA issues into the
                weight stream (used to prefetch quarter 3's resident tile)."""
                cols = slice(q * OQ, (q + 1) * OQ)
                bank = (q % 2) * 4
                accs = [
                    pp.tile([P, OQ], F32, name=f"acc{q}_{i}", tag=f"pb{bank + i}")
                    for i in range(NTC)
                ]
                if up_first:
                    for i in range(NTC):
                        ts = slice(i * P, (i + 1) * P)
                        nc.tensor.matmul(
                            accs[i][:], lhsT=twT_sb[:, ts],
                            rhs=bcat_sb[:, cols], start=True, stop=False,
                        )
                for kc in range(KT // KC):
                    ks = slice(kc * KC, (kc + 1) * KC)
                    wh_t = w_pool.tile([P, KC, OQ], F16, name="wh_t", tag="wh_t")
                    nc.sync.dma_start(out=wh_t[:], in_=wh_r[:, ks, cols])
                    if extra_dmas and kc in extra_dmas:
                        extra_dmas[kc]()
                    for kk in range(KC):
                        k = kc * KC + kk
                        for i in range(NTC):
                            ts = slice(i * P, (i + 1) * P)
                            nc.tensor.matmul(
                                accs[i][:], lhsT=xh_sb[:, k, ts],
                                rhs=wh_t[:, kk, :],
                                start=(k == 0 and not up_first),
                                stop=(k == KT - 1 and up_first),
                            )
                return accs

            def quarter3_accmajor(wh3_sb):
                """Final O-quarter, token-chunk-major: each acc opens with the
                lora up matmul, runs all 16 k's, and evicts immediately — the
                drain overlaps the remaining accs' matmuls instead of
                serializing at the end. Needs the quarter's weights resident."""
                cols = slice(3 * OQ, 4 * OQ)
                for i in range(NTC):
                    ts = slice(i * P, (i + 1) * P)
                    acc = pp.tile([P, OQ], F32, name=f"acc3_{i}", tag=f"pb{4 + i}")
                    nc.tensor.matmul(
                        acc[:], lhsT=twT_sb[:, ts], rhs=bcat_sb[:, cols],
                        start=True, stop=False,
                    )
                    for k in range(KT):
                        nc.tensor.matmul(
                            acc[:], lhsT=xh_sb[:, k, ts], rhs=wh3_sb[:, k, :],
                            start=False, stop=(k == KT - 1),
                        )
                    o_t = o_pool.tile([P, OQ], F16, name="o_t", tag="o_t")
                    nc.vector.tensor_copy(o_t[:], acc[:])
                    nc.scalar.dma_start(
                        out=out[i * P : (i + 1) * P, 3 * OQ : 4 * OQ], in_=o_t[:],
                    )

            def up_close(q, accs):
                """Close each accumulation group with the lora up matmul."""
                for i in range(NTC):
                    ts = slice(i * P, (i + 1) * P)
                    nc.tensor.matmul(
                        accs[i][:], lhsT=twT_sb[:, ts],
                        rhs=bcat_sb[:, q * OQ : (q + 1) * OQ],
                        start=False, stop=True,
                    )

            def evict(q, accs):
                for i in range(NTC):
                    o_t = o_pool.tile([P, OQ], F16, name="o_t", tag="o_t")
                    # DVE copies only: ACT must stay free to trigger its
                    # HWDGE DMA ring without queueing behind slow copies
                    nc.vector.tensor_copy(o_t[:], accs[i][:])
                    nc.scalar.dma_start(
                        out=out[i * P : (i + 1) * P, q * OQ : (q + 1) * OQ],
                        in_=o_t[:],
                    )

            def router_math(ga_tiles):
                """Batched top-2 routing for all 4 token chunks at once.
                ga_tiles[i][:, ER:GA] are the logits [t=128, e=8]."""
                l_all = r_pool.tile([P, NTC, E], F32, name="l_all")
                for i in range(NTC):
                    nc.vector.tensor_copy(l_all[:, i, :], ga_tiles[i][:, ER:GA])
                m1 = r_pool.tile([P, NTC], F32, name="m1")
                nc.vector.reduce_max(out=m1[:], in_=l_all[:], axis=AX)

                def bcast(ap):  # [P, NTC] -> [P, NTC, E]
                    return ap.rearrange("p c -> p c ()").broadcast_to([P, NTC, E])

                is1 = r_pool.tile([P, NTC, E], F32, name="is1")
                nc.vector.tensor_tensor(
                    out=is1[:], in0=l_all[:], in1=bcast(m1[:]), op=OP.is_equal
                )
                l2 = r_pool.tile([P, NTC, E], F32, name="l2")
                nc.vector.tensor_scalar(
                    out=l2[:], in0=is1[:], scalar1=-NEG_BIG, scalar2=None,
                    op0=OP.mult,
                )
                nc.vector.tensor_add(out=l2[:], in0=l2[:], in1=l_all[:])
                m2 = r_pool.tile([P, NTC], F32, name="m2")
                nc.vector.reduce_max(out=m2[:], in_=l2[:], axis=AX)
                is2 = r_pool.tile([P, NTC, E], F32, name="is2")
                nc.vector.tensor_tensor(
                    out=is2[:], in0=l2[:], in1=bcast(m2[:]), op=OP.is_equal
                )
                # s1 = sigmoid(m1 - m2) on ACT; s2 = 1 - s1 via sigmoid(-d)
                d12 = r_pool.tile([P, NTC], F32, name="d12")
                nc.vector.tensor_sub(out=d12[:], in0=m1[:], in1=m2[:])
                s1 = r_pool.tile([P, NTC], F32, name="s1")
                nc.scalar.activation(s1[:], d12[:], mybir.ActivationFunctionType.Sigmoid)
                s2 = r_pool.tile([P, NTC], F32, name="s2")
                nc.scalar.activation(
                    s2[:], d12[:], mybir.ActivationFunctionType.Sigmoid, scale=-1.0
                )
                cw = r_pool.tile([P, NTC, E], F32, name="cw")
                nc.vector.tensor_tensor(
                    out=cw[:], in0=is1[:], in1=bcast(s1[:]), op=OP.mult
                )
                cw2 = r_pool.tile([P, NTC, E], F32, name="cw2")
                nc.vector.tensor_tensor(
                    out=cw2[:], in0=is2[:], in1=bcast(s2[:]), op=OP.mult
                )
                nc.vector.tensor_add(out=cw[:], in0=cw[:], in1=cw2[:])

                # tw[t, (e r)] = t_down[t, (e r)] * cw[t, e]; transpose to
                # [er, t] for use as the up-projection stationary operand.
                for i in range(NTC):
                    ts = slice(i * P, (i + 1) * P)
                    tw_sb = r_pool.tile([P, ER], F16, name=f"tw_sb{i}", tag="tw_sb")
                    nc.vector.tensor_tensor(
                        out=tw_sb[:].rearrange("p (e r) -> p e r", r=R),
                        in0=ga_tiles[i][:, 0:ER].rearrange("p (e r) -> p e r", r=R),
                        in1=cw[:, i, :].rearrange("p e -> p e ()").broadcast_to(
                            [P, E, R]
                        ),
                        op=OP.mult,
                    )
                    twT_ps = pp.tile([P, P], F16, name=f"twT_ps{i}", tag=f"pb{4 + i}")
                    nc.tensor.transpose(twT_ps[:], tw_sb[:], identity[:])
                    nc.vector.tensor_copy(twT_sb[:, ts], twT_ps[:])

            # ---- program ----
            # ga_ps[t, 0:128] = lora-down t; ga_ps[t, 128:136] = router logits.
            ga_tiles = [
                pp.tile([P, GA], F32, name=f"ga_ps{i}", tag=f"pb{4 + i}")
                for i in range(NTC)
            ]
            accs0 = quarter0(ga_tiles)
            router_math(ga_tiles)                # DVE/ACT; frees pb4-7
            accs1 = base_quarter(1, up_first=False)  # pb4-7, starts right away
            up_close(0, accs0)                   # twT ready by now
            evict(0, accs0)
            # quarter 3's weights prefetch into a resident tile, interleaved
            # with quarter 2's stream on the SP ring
            wh3_sb = big_pool.tile([P, KT, OQ], F16)
            c3 = slice(3 * OQ, 4 * OQ)
            extra = {
                1: lambda: nc.sync.dma_start(out=wh3_sb[:, 0:4, :],
                                             in_=wh_r[:, 0:4, c3]),
                2: lambda: nc.sync.dma_start(out=wh3_sb[:, 4:8, :],
                                             in_=wh_r[:, 4:8, c3]),
                3: lambda: (nc.sync.dma_start(out=wh3_sb[:, 8:12, :],
                                              in_=wh_r[:, 8:12, c3]),
                            nc.sync.dma_start(out=wh3_sb[:, 12:16, :],
                                              in_=wh_r[:, 12:16, c3])),
            }
            accs2 = base_quarter(2, up_first=True, extra_dmas=extra)
            up_close(1, accs1)
            evict(1, accs1)
            evict(2, accs2)
            quarter3_accmajor(wh3_sb)            # pb4-7; evicts inline

    nc.compile()
    return nc


_NC_CACHE = {}


def _get_nc():
    if "nc" not in _NC_CACHE:
        _NC_CACHE["nc"] = _build_nc()
    return _NC_CACHE["nc"]


def _prep_in_maps(x, weight, gate_w, A_w, B_w):
    xf = np.asarray(x, np.float32).reshape(TOKENS, H)
    wh = np.ascontiguousarray(np.asarray(weight, np.float32).T).astype(np.float16)
    acatT = np.asarray(A_w, np.float32).transpose(2, 0, 1).reshape(H, ER)
    gah = np.ascontiguousarray(
        np.concatenate([acatT, np.asarray(gate_w, np.float32).T], axis=1)
    ).astype(np.float16)
    bcat = np.ascontiguousarray(
        (np.asarray(B_w, np.float32).transpose(0, 2, 1).reshape(ER, O) * LORA_ALPHA)
        .astype(np.float16)
    )
    shared = {"wh": wh, "gah": gah, "bcat": bcat}
    in_maps = []
    for c in range(NCORES):
        xch = np.ascontiguousarray(xf[c * T : (c + 1) * T, :].T).astype(np.float16)
        in_maps.append({"xh": xch, **shared})
    return in_maps


def kernel(x, weight, gate_w, A_w, B_w, _trace=False, **_ignored):
    in_maps = _prep_in_maps(x, weight, gate_w, A_w, B_w)
    nc = _get_nc()
    res = bass_utils.run_bass_kernel_spmd(
        nc, in_maps, core_ids=list(range(NCORES)), trace=_trace
    )
    outs = [res.results[c]["out"] for c in range(NCORES)]
    full = np.concatenate(outs, axis=0).reshape(B, S, O).astype(np.float32)
    if _trace:
        kernel.last_result = res
    return full
